# revision 4
# baseline (speedup 1.0000x reference)
"""DAAutoencoder (2-layer DAA net: masked min then masked max) on 8 TRN2 cores.

Math (reference):
  h[b,o]   = min over {i: sel0[o,i]==1} of x[b,i]   (2.0 if no edge)
  out[b,j] = max over {o: sel1[j,o]==1} of h[b,o]   (-1.0 if no edge)

Kernel formulation (exact, selection-only — bitwise identical to reference):
  Work in the negated domain for layer 0 so both layers reduce with max:
    -h[b,o]  = max( -2.0, max_i( (-x[b,i]) + nb0[i,o] ) ),  nb0 = 0 if edge else -4
    out[b,j] = max( -1.0, max_o( (-(-h[b,o])) + b1[o,j] ) ), b1 = 0 if edge else -4
  Masked-out lanes land below the init value and never win; winners are the
  original float values untouched (x + 0.0 == x exactly).

Data-parallel over batch: 256 rows -> 8 cores x 32 rows. sel-derived bias
tables replicated. Per core, per batch row b:
  layer 0: 8x scalar_tensor_tensor [128i,512o]  (op0=add per-partition -x col,
           op1=max into running acc) -> gpsimd partition_all_reduce(max)
           -> row copy -> (-h) rows [32,512]
  PE-transpose (-h) rows -> (-h) column tiles [128o,32b]
  layer 1: 4x scalar_tensor_tensor [128o,1024j] (op0=subtract (-h) col == add h,
           op1=max) -> partition_all_reduce(max) -> row copy -> out rows
Reduction dim always sits on partitions; the per-(b) vector enters as the
per-partition scalar operand of scalar_tensor_tensor, so nothing is ever
broadcast across partitions.
"""

import numpy as np

import concourse.bacc as bacc
import concourse.mybir as mybir
from concourse import tile, masks
from concourse.bass_isa import ReduceOp
from concourse import bass_utils

B, IN, HID = 256, 1024, 512
NCORES = 8
BS = B // NCORES  # 32 batch rows per core

_cache = {}


def _build():
    nc = bacc.Bacc("TRN2", target_bir_lowering=False, debug=False,
                   num_devices=NCORES)
    f32 = mybir.dt.float32
    xtn = nc.dram_tensor("xtn", [IN, BS], f32, kind="ExternalInput")
    nb0t = nc.dram_tensor("nb0t", [IN, HID], f32, kind="ExternalInput")
    b1t = nc.dram_tensor("b1t", [HID, IN], f32, kind="ExternalInput")
    out = nc.dram_tensor("out", [BS, IN], f32, kind="ExternalOutput")

    NT0 = IN // 128   # 8 i-tiles for layer 0
    NT1 = HID // 128  # 4 o-tiles for layer 1

    with tile.TileContext(nc) as tc:
        with tc.tile_pool(name="tables", bufs=1) as tabs, \
             tc.tile_pool(name="work", bufs=3) as work, \
             tc.tile_pool(name="res", bufs=1) as resp, \
             tc.tile_pool(name="psum", bufs=4, space="PSUM") as psp:

            nb0 = []
            for t in range(NT0):
                tt = tabs.tile([128, HID], f32, tag=f"nb0_{t}")
                nc.sync.dma_start(tt[:], nb0t[128 * t:128 * (t + 1), :])
                nb0.append(tt)
            b1 = []
            for u in range(NT1):
                tt = tabs.tile([128, IN], f32, tag=f"b1_{u}")
                nc.sync.dma_start(tt[:], b1t[128 * u:128 * (u + 1), :])
                b1.append(tt)
            xc = []
            for t in range(NT0):
                tt = tabs.tile([128, BS], f32, tag=f"x_{t}")
                nc.sync.dma_start(tt[:], xtn[128 * t:128 * (t + 1), :])
                xc.append(tt)

            init0 = tabs.tile([128, HID], f32, tag="init0")
            nc.vector.memset(init0[:], -2.0)
            init1 = tabs.tile([128, IN], f32, tag="init1")
            nc.vector.memset(init1[:], -1.0)
            ident = tabs.tile([128, 128], f32, tag="ident")
            masks.make_identity(nc, ident[:])

            hrows = resp.tile([BS, HID], f32, tag="hrows")     # rows of -h

            # ---- layer 0 ----
            for b in range(BS):
                r = work.tile([128, HID], f32, tag="r0")
                nc.vector.scalar_tensor_tensor(
                    r[:], nb0[0][:], xc[0][:, b:b + 1], init0[:],
                    op0=mybir.AluOpType.add, op1=mybir.AluOpType.max)
                for t in range(1, NT0):
                    nc.vector.scalar_tensor_tensor(
                        r[:], nb0[t][:], xc[t][:, b:b + 1], r[:],
                        op0=mybir.AluOpType.add, op1=mybir.AluOpType.max)
                p0 = work.tile([128, HID], f32, tag="p0")
                nc.gpsimd.partition_all_reduce(p0[:], r[:], 128, ReduceOp.max)
                # all partitions of p0 hold the same reduced row; DMA one out
                # (engines can't start at arbitrary partitions, DMA can write them)
                nc.sync.dma_start(hrows[b:b + 1, :], p0[0:1, :])

            # ---- transpose -h rows into column tiles [128, BS] ----
            hcols = []
            for c in range(NT1):
                ps = psp.tile([128, BS], f32, tag="ps_t")
                nc.tensor.transpose(
                    ps[:], hrows[:, 128 * c:128 * (c + 1)], ident[:BS, :BS])
                hc = tabs.tile([128, BS], f32, tag=f"hc_{c}")
                nc.scalar.copy(hc[:], ps[:])
                hcols.append(hc)

            # ---- layer 1 ----
            for b in range(BS):
                r = work.tile([128, IN], f32, tag="r1")
                nc.vector.scalar_tensor_tensor(
                    r[:], b1[0][:], hcols[0][:, b:b + 1], init1[:],
                    op0=mybir.AluOpType.subtract, op1=mybir.AluOpType.max)
                for u in range(1, NT1):
                    nc.vector.scalar_tensor_tensor(
                        r[:], b1[u][:], hcols[u][:, b:b + 1], r[:],
                        op0=mybir.AluOpType.subtract, op1=mybir.AluOpType.max)
                p1 = work.tile([128, IN], f32, tag="p1")
                nc.gpsimd.partition_all_reduce(p1[:], r[:], 128, ReduceOp.max)
                nc.sync.dma_start(out[b:b + 1, :], p1[0:1, :])
    nc.compile()
    return nc


def kernel(x, sel0, sel1):
    x = np.ascontiguousarray(np.asarray(x, dtype=np.float32))
    nb0t = np.where(np.asarray(sel0).T == 1, 0.0, -4.0).astype(np.float32)
    b1t = np.where(np.asarray(sel1).T == 1, 0.0, -4.0).astype(np.float32)
    nb0t = np.ascontiguousarray(nb0t)  # [IN, HID]
    b1t = np.ascontiguousarray(b1t)    # [HID, IN]

    if "nc" not in _cache:
        _cache["nc"] = _build()
    nc = _cache["nc"]

    in_maps = []
    for c in range(NCORES):
        xs = x[c * BS:(c + 1) * BS, :]             # [BS, IN]
        xtn = np.ascontiguousarray((-xs).T)        # [IN, BS]
        in_maps.append({"xtn": xtn, "nb0t": nb0t, "b1t": b1t})

    res = bass_utils.run_bass_kernel_spmd(
        nc, in_maps, core_ids=list(range(NCORES)))
    return np.concatenate([res.results[c]["out"] for c in range(NCORES)],
                          axis=0)


# revision 5
# speedup vs baseline: 1.3272x; 1.3272x over previous
"""DAAutoencoder (2-layer DAA net: masked min then masked max) on 8 TRN2 cores.

Math (reference):
  h[b,o]   = min over {i: sel0[o,i]==1} of x[b,i]   (2.0 if no edge)
  out[b,j] = max over {o: sel1[j,o]==1} of h[b,o]   (-1.0 if no edge)

Kernel formulation (exact, selection-only — bitwise identical to reference):
  Work in the negated domain for layer 0 so both layers reduce with max:
    -h[b,o]  = max( -2.0, max_i( (-x[b,i]) + nb0[i,o] ) ),  nb0 = 0 if edge else -4
    out[b,j] = max( -1.0, max_o( (-(-h[b,o])) + b1[o,j] ) ), b1 = 0 if edge else -4
  Masked-out lanes land below the init value and never win; winners are the
  original float values untouched (x + 0.0 == x exactly).

Data-parallel over batch: 256 rows -> 8 cores x 32 rows. sel-derived bias
tables replicated. Per core, per batch row b:
  layer 0: 8x scalar_tensor_tensor [128i,512o]  (op0=add per-partition -x col,
           op1=max into running acc) -> gpsimd partition_all_reduce(max)
           -> row copy -> (-h) rows [32,512]
  PE-transpose (-h) rows -> (-h) column tiles [128o,32b]
  layer 1: 4x scalar_tensor_tensor [128o,1024j] (op0=subtract (-h) col == add h,
           op1=max) -> partition_all_reduce(max) -> row copy -> out rows
Reduction dim always sits on partitions; the per-(b) vector enters as the
per-partition scalar operand of scalar_tensor_tensor, so nothing is ever
broadcast across partitions.
"""

import numpy as np

import concourse.bacc as bacc
import concourse.mybir as mybir
from concourse import tile, masks
from concourse.bass_isa import ReduceOp
from concourse import bass_utils

B, IN, HID = 256, 1024, 512
NCORES = 8
BS = B // NCORES  # 32 batch rows per core

_cache = {}


def _build():
    nc = bacc.Bacc("TRN2", target_bir_lowering=False, debug=False,
                   num_devices=NCORES)
    f32 = mybir.dt.float32
    xtn = nc.dram_tensor("xtn", [IN, BS], f32, kind="ExternalInput")
    nb0t = nc.dram_tensor("nb0t", [IN, HID], f32, kind="ExternalInput")
    b1t = nc.dram_tensor("b1t", [HID, IN], f32, kind="ExternalInput")
    out = nc.dram_tensor("out", [BS, IN], f32, kind="ExternalOutput")

    NT0 = IN // 128   # 8 i-tiles for layer 0
    NT1 = HID // 128  # 4 o-tiles for layer 1

    with tile.TileContext(nc) as tc:
        with tc.tile_pool(name="tables", bufs=1) as tabs, \
             tc.tile_pool(name="work", bufs=3) as work, \
             tc.tile_pool(name="res", bufs=1) as resp, \
             tc.tile_pool(name="psum", bufs=4, space="PSUM") as psp:

            nb0 = []
            for t in range(NT0):
                tt = tabs.tile([128, HID], f32, tag=f"nb0_{t}")
                nc.sync.dma_start(tt[:], nb0t[128 * t:128 * (t + 1), :])
                nb0.append(tt)
            b1 = []
            for u in range(NT1):
                tt = tabs.tile([128, IN], f32, tag=f"b1_{u}")
                nc.sync.dma_start(tt[:], b1t[128 * u:128 * (u + 1), :])
                b1.append(tt)
            xc = []
            for t in range(NT0):
                tt = tabs.tile([128, BS], f32, tag=f"x_{t}")
                nc.sync.dma_start(tt[:], xtn[128 * t:128 * (t + 1), :])
                xc.append(tt)

            init0 = tabs.tile([128, HID], f32, tag="init0")
            nc.vector.memset(init0[:], -2.0)
            init1 = tabs.tile([128, IN], f32, tag="init1")
            nc.vector.memset(init1[:], -1.0)
            ident = tabs.tile([128, 128], f32, tag="ident")
            masks.make_identity(nc, ident[:])

            hrows = resp.tile([BS, HID], f32, tag="hrows")     # rows of -h

            # ---- layer 0 ----
            for b in range(BS):
                r = work.tile([128, HID], f32, tag="r0")
                nc.vector.scalar_tensor_tensor(
                    r[:], nb0[0][:], xc[0][:, b:b + 1], init0[:],
                    op0=mybir.AluOpType.add, op1=mybir.AluOpType.max)
                for t in range(1, NT0):
                    nc.vector.scalar_tensor_tensor(
                        r[:], nb0[t][:], xc[t][:, b:b + 1], r[:],
                        op0=mybir.AluOpType.add, op1=mybir.AluOpType.max)
                p0 = work.tile([128, HID], f32, tag="p0")
                nc.gpsimd.partition_all_reduce(p0[:], r[:], 128, ReduceOp.max)
                # all partitions of p0 hold the same reduced row; DMA one out
                # (engines can't start at arbitrary partitions, DMA can write them)
                nc.sync.dma_start(hrows[b:b + 1, :], p0[0:1, :])

            # ---- transpose -h rows into column tiles [128, BS] ----
            hcols = []
            for c in range(NT1):
                ps = psp.tile([128, BS], f32, tag="ps_t")
                nc.tensor.transpose(
                    ps[:], hrows[:, 128 * c:128 * (c + 1)], ident[:BS, :BS])
                hc = tabs.tile([128, BS], f32, tag=f"hc_{c}")
                nc.scalar.copy(hc[:], ps[:])
                hcols.append(hc)

            # ---- layer 1 ----
            for b in range(BS):
                r = work.tile([128, IN], f32, tag="r1")
                nc.vector.scalar_tensor_tensor(
                    r[:], b1[0][:], hcols[0][:, b:b + 1], init1[:],
                    op0=mybir.AluOpType.subtract, op1=mybir.AluOpType.max)
                for u in range(1, NT1):
                    nc.vector.scalar_tensor_tensor(
                        r[:], b1[u][:], hcols[u][:, b:b + 1], r[:],
                        op0=mybir.AluOpType.subtract, op1=mybir.AluOpType.max)
                p1 = work.tile([128, IN], f32, tag="p1")
                nc.gpsimd.partition_all_reduce(p1[:], r[:], 128, ReduceOp.max)
                nc.sync.dma_start(out[b:b + 1, :], p1[0:1, :])
    nc.compile()
    return nc


def _make_in_maps(x, sel0, sel1):
    x = np.ascontiguousarray(np.asarray(x, dtype=np.float32))
    nb0t = np.ascontiguousarray(
        np.where(np.asarray(sel0).T == 1, 0.0, -4.0).astype(np.float32))
    b1t = np.ascontiguousarray(
        np.where(np.asarray(sel1).T == 1, 0.0, -4.0).astype(np.float32))
    in_maps = []
    for c in range(NCORES):
        xs = x[c * BS:(c + 1) * BS, :]             # [BS, IN]
        xtn = np.ascontiguousarray((-xs).T)        # [IN, BS]
        in_maps.append({"xtn": xtn, "nb0t": nb0t, "b1t": b1t})
    return in_maps


def _fast_runner(nc):
    """Build a reusable jitted runner (same plumbing as
    bass2jax.run_bass_via_pjrt, but the jitted callable is cached so repeat
    kernel() calls don't re-trace/re-compile)."""
    import jax
    from jax.sharding import Mesh, PartitionSpec
    from jax.experimental.shard_map import shard_map
    import concourse.mybir as mb
    from concourse.bass2jax import (_bass_exec_p, install_neuronx_cc_hook,
                                    partition_id_tensor)

    install_neuronx_cc_hook()
    partition_name = (nc.partition_id_tensor.name
                      if nc.partition_id_tensor else None)
    in_names, out_names, out_avals = [], [], []
    for alloc in nc.m.functions[0].allocations:
        if not isinstance(alloc, mb.MemoryLocationSet):
            continue
        name = alloc.memorylocations[0].name
        if alloc.kind == "ExternalInput":
            if name != partition_name:
                in_names.append(name)
        elif alloc.kind == "ExternalOutput":
            out_names.append(name)
            out_avals.append(jax.core.ShapedArray(
                tuple(alloc.tensor_shape), mb.dt.np(alloc.dtype)))
    n_params = len(in_names)
    n_outs = len(out_avals)
    all_in_names = list(in_names) + list(out_names)
    if partition_name is not None:
        all_in_names.append(partition_name)
    donate = tuple(range(n_params, n_params + n_outs))

    def _body(*args):
        operands = list(args)
        if partition_name is not None:
            operands.append(partition_id_tensor())
        return tuple(_bass_exec_p.bind(
            *operands, out_avals=tuple(out_avals), in_names=tuple(all_in_names),
            out_names=tuple(out_names), lowering_input_output_aliases=(),
            sim_require_finite=True, sim_require_nnan=True, nc=nc))

    devices = jax.devices()[:NCORES]
    mesh = Mesh(np.asarray(devices), ("core",))
    sharded = jax.jit(
        shard_map(_body, mesh=mesh,
                  in_specs=(PartitionSpec("core"),) * (n_params + n_outs),
                  out_specs=(PartitionSpec("core"),) * n_outs,
                  check_rep=False),
        donate_argnums=donate, keep_unused=True)

    def run(in_maps):
        concat_in = [
            np.concatenate([np.asarray(in_maps[c][nm]) for c in range(NCORES)],
                           axis=0)
            for nm in in_names]
        concat_zeros = [
            np.zeros((NCORES * a.shape[0], *a.shape[1:]), a.dtype)
            for a in out_avals]
        out_arrs = sharded(*concat_in, *concat_zeros)
        return [
            {nm: np.asarray(out_arrs[i]).reshape(NCORES, *out_avals[i].shape)[c]
             for i, nm in enumerate(out_names)}
            for c in range(NCORES)]

    return run


def kernel(x, sel0, sel1):
    in_maps = _make_in_maps(x, sel0, sel1)
    if "nc" not in _cache:
        _cache["nc"] = _build()
        # first call goes through the stock runner (compiles the NEFF)
        res = bass_utils.run_bass_kernel_spmd(
            _cache["nc"], in_maps, core_ids=list(range(NCORES)))
        results = res.results
        _cache["run"] = _fast_runner(_cache["nc"])
    else:
        results = _cache["run"](in_maps)
    return np.concatenate([results[c]["out"] for c in range(NCORES)], axis=0)


# revision 6
# speedup vs baseline: 1.8849x; 1.4202x over previous
"""DAAutoencoder (2-layer DAA net: masked min then masked max) on 8 TRN2 cores.

Math (reference):
  h[b,o]   = min over {i: sel0[o,i]==1} of x[b,i]   (2.0 if no edge)
  out[b,j] = max over {o: sel1[j,o]==1} of h[b,o]   (-1.0 if no edge)

Kernel formulation (exact, selection-only — bitwise identical to reference):
  Work in the negated domain for layer 0 so both layers reduce with max:
    -h[b,o]  = max( -2.0, max_i( (-x[b,i]) + nb0[i,o] ) ),  nb0 = 0 if edge else -4
    out[b,j] = max( -1.0, max_o( (-(-h[b,o])) + b1[o,j] ) ), b1 = 0 if edge else -4
  Masked-out lanes land below the init value and never win; winners are the
  original float values untouched (x + 0.0 == x exactly).

Data-parallel over batch: 256 rows -> 8 cores x 32 rows. sel-derived bias
tables replicated. Per core, per batch row b:
  layer 0: 8x scalar_tensor_tensor [128i,512o]  (op0=add per-partition -x col,
           op1=max into running acc) -> gpsimd partition_all_reduce(max)
           -> row copy -> (-h) rows [32,512]
  PE-transpose (-h) rows -> (-h) column tiles [128o,32b]
  layer 1: 4x scalar_tensor_tensor [128o,1024j] (op0=subtract (-h) col == add h,
           op1=max) -> partition_all_reduce(max) -> row copy -> out rows
Reduction dim always sits on partitions; the per-(b) vector enters as the
per-partition scalar operand of scalar_tensor_tensor, so nothing is ever
broadcast across partitions.
"""

import numpy as np

import concourse.bacc as bacc
import concourse.mybir as mybir
from concourse import tile, masks
from concourse.bass_isa import ReduceOp
from concourse import bass_utils

B, IN, HID = 256, 1024, 512
NCORES = 8
BS = B // NCORES  # 32 batch rows per core

_cache = {}


def _build():
    nc = bacc.Bacc("TRN2", target_bir_lowering=False, debug=False,
                   num_devices=NCORES)
    f32 = mybir.dt.float32
    xtn = nc.dram_tensor("xtn", [IN, BS], f32, kind="ExternalInput")
    nb0t = nc.dram_tensor("nb0t", [IN, HID], f32, kind="ExternalInput")
    b1t = nc.dram_tensor("b1t", [HID, IN], f32, kind="ExternalInput")
    out = nc.dram_tensor("out", [BS, IN], f32, kind="ExternalOutput")

    NT0 = IN // 128   # 8 i-tiles for layer 0
    NT1 = HID // 128  # 4 o-tiles for layer 1

    with tile.TileContext(nc) as tc:
        with tc.tile_pool(name="tables", bufs=1) as tabs, \
             tc.tile_pool(name="work", bufs=3) as work, \
             tc.tile_pool(name="res", bufs=1) as resp, \
             tc.tile_pool(name="psum", bufs=4, space="PSUM") as psp:

            nb0 = []
            for t in range(NT0):
                tt = tabs.tile([128, HID], f32, tag=f"nb0_{t}")
                nc.sync.dma_start(tt[:], nb0t[128 * t:128 * (t + 1), :])
                nb0.append(tt)
            b1 = []
            for u in range(NT1):
                tt = tabs.tile([128, IN], f32, tag=f"b1_{u}")
                nc.sync.dma_start(tt[:], b1t[128 * u:128 * (u + 1), :])
                b1.append(tt)
            xc = []
            for t in range(NT0):
                tt = tabs.tile([128, BS], f32, tag=f"x_{t}")
                nc.sync.dma_start(tt[:], xtn[128 * t:128 * (t + 1), :])
                xc.append(tt)

            init0 = tabs.tile([128, HID], f32, tag="init0")
            nc.vector.memset(init0[:], -2.0)
            init1 = tabs.tile([128, IN], f32, tag="init1")
            nc.vector.memset(init1[:], -1.0)
            ident = tabs.tile([128, 128], f32, tag="ident")
            masks.make_identity(nc, ident[:])

            hrows = resp.tile([BS, HID], f32, tag="hrows")     # rows of -h

            # ---- layer 0 ----
            for b in range(BS):
                r = work.tile([128, HID], f32, tag="r0")
                nc.vector.scalar_tensor_tensor(
                    r[:], nb0[0][:], xc[0][:, b:b + 1], init0[:],
                    op0=mybir.AluOpType.add, op1=mybir.AluOpType.max)
                for t in range(1, NT0):
                    nc.vector.scalar_tensor_tensor(
                        r[:], nb0[t][:], xc[t][:, b:b + 1], r[:],
                        op0=mybir.AluOpType.add, op1=mybir.AluOpType.max)
                p0 = work.tile([128, HID], f32, tag="p0")
                nc.gpsimd.partition_all_reduce(p0[:], r[:], 128, ReduceOp.max)
                # all partitions of p0 hold the same reduced row; DMA one out
                # (engines can't start at arbitrary partitions, DMA can write them)
                nc.sync.dma_start(hrows[b:b + 1, :], p0[0:1, :])

            # ---- transpose -h rows into column tiles [128, BS] ----
            hcols = []
            for c in range(NT1):
                ps = psp.tile([128, BS], f32, tag="ps_t")
                nc.tensor.transpose(
                    ps[:], hrows[:, 128 * c:128 * (c + 1)], ident[:BS, :BS])
                hc = tabs.tile([128, BS], f32, tag=f"hc_{c}")
                nc.scalar.copy(hc[:], ps[:])
                hcols.append(hc)

            # ---- layer 1 ----
            for b in range(BS):
                r = work.tile([128, IN], f32, tag="r1")
                nc.vector.scalar_tensor_tensor(
                    r[:], b1[0][:], hcols[0][:, b:b + 1], init1[:],
                    op0=mybir.AluOpType.subtract, op1=mybir.AluOpType.max)
                for u in range(1, NT1):
                    nc.vector.scalar_tensor_tensor(
                        r[:], b1[u][:], hcols[u][:, b:b + 1], r[:],
                        op0=mybir.AluOpType.subtract, op1=mybir.AluOpType.max)
                p1 = work.tile([128, IN], f32, tag="p1")
                nc.gpsimd.partition_all_reduce(p1[:], r[:], 128, ReduceOp.max)
                nc.sync.dma_start(out[b:b + 1, :], p1[0:1, :])
    nc.compile()
    return nc


def _make_in_maps(x, sel0, sel1):
    x = np.ascontiguousarray(np.asarray(x, dtype=np.float32))
    nb0t = np.ascontiguousarray(
        np.where(np.asarray(sel0).T == 1, 0.0, -4.0).astype(np.float32))
    b1t = np.ascontiguousarray(
        np.where(np.asarray(sel1).T == 1, 0.0, -4.0).astype(np.float32))
    in_maps = []
    for c in range(NCORES):
        xs = x[c * BS:(c + 1) * BS, :]             # [BS, IN]
        xtn = np.ascontiguousarray((-xs).T)        # [IN, BS]
        in_maps.append({"xtn": xtn, "nb0t": nb0t, "b1t": b1t})
    return in_maps


def _fast_runner(nc):
    """Build a reusable jitted runner (same plumbing as
    bass2jax.run_bass_via_pjrt, but the jitted callable is cached so repeat
    kernel() calls don't re-trace/re-compile)."""
    import jax
    from jax.sharding import Mesh, PartitionSpec
    from jax.experimental.shard_map import shard_map
    import concourse.mybir as mb
    from concourse.bass2jax import (_bass_exec_p, install_neuronx_cc_hook,
                                    partition_id_tensor)

    install_neuronx_cc_hook()
    partition_name = (nc.partition_id_tensor.name
                      if nc.partition_id_tensor else None)
    in_names, out_names, out_avals = [], [], []
    for alloc in nc.m.functions[0].allocations:
        if not isinstance(alloc, mb.MemoryLocationSet):
            continue
        name = alloc.memorylocations[0].name
        if alloc.kind == "ExternalInput":
            if name != partition_name:
                in_names.append(name)
        elif alloc.kind == "ExternalOutput":
            out_names.append(name)
            out_avals.append(jax.core.ShapedArray(
                tuple(alloc.tensor_shape), mb.dt.np(alloc.dtype)))
    n_params = len(in_names)
    n_outs = len(out_avals)
    all_in_names = list(in_names) + list(out_names)
    if partition_name is not None:
        all_in_names.append(partition_name)
    donate = tuple(range(n_params, n_params + n_outs))

    def _body(*args):
        operands = list(args)
        if partition_name is not None:
            operands.append(partition_id_tensor())
        return tuple(_bass_exec_p.bind(
            *operands, out_avals=tuple(out_avals), in_names=tuple(all_in_names),
            out_names=tuple(out_names), lowering_input_output_aliases=(),
            sim_require_finite=True, sim_require_nnan=True, nc=nc))

    devices = jax.devices()[:NCORES]
    mesh = Mesh(np.asarray(devices), ("core",))
    # per-core inputs (xtn) are sharded along axis 0; the sel-derived bias
    # tables are identical on every core -> replicate them (one H2D transfer
    # instead of 8 concatenated copies through the relay)
    repl = {"nb0t", "b1t"}
    in_specs = tuple(
        (PartitionSpec() if nm in repl else PartitionSpec("core"))
        for nm in in_names) + (PartitionSpec("core"),) * n_outs
    sharded = jax.jit(
        shard_map(_body, mesh=mesh, in_specs=in_specs,
                  out_specs=(PartitionSpec("core"),) * n_outs,
                  check_rep=False),
        donate_argnums=donate, keep_unused=True)

    def run(in_maps):
        concat_in = [
            np.asarray(in_maps[0][nm]) if nm in repl else
            np.concatenate([np.asarray(in_maps[c][nm]) for c in range(NCORES)],
                           axis=0)
            for nm in in_names]
        concat_zeros = [
            np.zeros((NCORES * a.shape[0], *a.shape[1:]), a.dtype)
            for a in out_avals]
        out_arrs = sharded(*concat_in, *concat_zeros)
        return [
            {nm: np.asarray(out_arrs[i]).reshape(NCORES, *out_avals[i].shape)[c]
             for i, nm in enumerate(out_names)}
            for c in range(NCORES)]

    return run


def kernel(x, sel0, sel1):
    in_maps = _make_in_maps(x, sel0, sel1)
    if "nc" not in _cache:
        _cache["nc"] = _build()
        # first call goes through the stock runner (compiles the NEFF)
        res = bass_utils.run_bass_kernel_spmd(
            _cache["nc"], in_maps, core_ids=list(range(NCORES)))
        results = res.results
        _cache["run"] = _fast_runner(_cache["nc"])
    else:
        results = _cache["run"](in_maps)
    return np.concatenate([results[c]["out"] for c in range(NCORES)], axis=0)


# revision 9
# speedup vs baseline: 324.6275x; 172.2234x over previous
"""DAAutoencoder (2-layer DAA net: masked min then masked max) on 8 TRN2 cores.

Math (reference):
  h[b,o]   = min over {i: sel0[o,i]==1} of x[b,i]   (2.0 if no edge)
  out[b,j] = max over {o: sel1[j,o]==1} of h[b,o]   (-1.0 if no edge)

Kernel formulation (exact, selection-only — bitwise identical to reference):
  Work in the negated domain for layer 0 so both layers reduce with max:
    -h[b,o]  = max( -2.0, max_i( (-x[b,i]) + nb0[i,o] ) ),  nb0 = 0 if edge else -4
    out[b,j] = max( -1.0, max_o( (-(-h[b,o])) + b1[o,j] ) ), b1 = 0 if edge else -4
  Masked-out lanes land below the init value and never win; winners are the
  original float values untouched (x + 0.0 == x exactly).

Data-parallel over batch: 256 rows -> 8 cores x 32 rows. sel-derived bias
tables replicated. Per core, per batch row b:
  layer 0: 8x scalar_tensor_tensor [128i,512o]  (op0=add per-partition -x col,
           op1=max into running acc) -> gpsimd partition_all_reduce(max)
           -> row copy -> (-h) rows [32,512]
  PE-transpose (-h) rows -> (-h) column tiles [128o,32b]
  layer 1: 4x scalar_tensor_tensor [128o,1024j] (op0=subtract (-h) col == add h,
           op1=max) -> partition_all_reduce(max) -> row copy -> out rows
Reduction dim always sits on partitions; the per-(b) vector enters as the
per-partition scalar operand of scalar_tensor_tensor, so nothing is ever
broadcast across partitions.
"""

import numpy as np

import concourse.bacc as bacc
import concourse.mybir as mybir
from concourse import tile, masks
from concourse.bass_isa import ReduceOp
from concourse import bass_utils

B, IN, HID = 256, 1024, 512
NCORES = 8
BS = B // NCORES  # 32 batch rows per core

_cache = {}


def _build(repeat=1):
    nc = bacc.Bacc("TRN2", target_bir_lowering=False, debug=False,
                   num_devices=NCORES)
    f32 = mybir.dt.float32
    xtn = nc.dram_tensor("xtn", [IN, BS], f32, kind="ExternalInput")
    nb0t = nc.dram_tensor("nb0t", [IN, HID], f32, kind="ExternalInput")
    b1t = nc.dram_tensor("b1t", [HID, IN], f32, kind="ExternalInput")
    out = nc.dram_tensor("out", [BS, IN], f32, kind="ExternalOutput")

    NT0 = IN // 128   # 8 i-tiles for layer 0
    NT1 = HID // 128  # 4 o-tiles for layer 1

    with tile.TileContext(nc) as tc:
        with tc.tile_pool(name="tables", bufs=1) as tabs, \
             tc.tile_pool(name="work", bufs=3) as work, \
             tc.tile_pool(name="res", bufs=1) as resp, \
             tc.tile_pool(name="psum", bufs=4, space="PSUM") as psp:

            nb0 = []
            for t in range(NT0):
                tt = tabs.tile([128, HID], f32, tag=f"nb0_{t}")
                nc.sync.dma_start(tt[:], nb0t[128 * t:128 * (t + 1), :])
                nb0.append(tt)
            b1 = []
            for u in range(NT1):
                tt = tabs.tile([128, IN], f32, tag=f"b1_{u}")
                nc.sync.dma_start(tt[:], b1t[128 * u:128 * (u + 1), :])
                b1.append(tt)
            xc = []
            for t in range(NT0):
                tt = tabs.tile([128, BS], f32, tag=f"x_{t}")
                nc.sync.dma_start(tt[:], xtn[128 * t:128 * (t + 1), :])
                xc.append(tt)

            init0 = tabs.tile([128, HID], f32, tag="init0")
            nc.vector.memset(init0[:], -2.0)
            init1 = tabs.tile([128, IN], f32, tag="init1")
            nc.vector.memset(init1[:], -1.0)
            ident = tabs.tile([128, 128], f32, tag="ident")
            masks.make_identity(nc, ident[:])

            hrows = resp.tile([BS, HID], f32, tag="hrows")     # rows of -h

            for _rep in range(repeat):
              # ---- layer 0 ----
              for b in range(BS):
                r = work.tile([128, HID], f32, tag="r0")
                nc.vector.scalar_tensor_tensor(
                    r[:], nb0[0][:], xc[0][:, b:b + 1], init0[:],
                    op0=mybir.AluOpType.add, op1=mybir.AluOpType.max)
                for t in range(1, NT0):
                    nc.vector.scalar_tensor_tensor(
                        r[:], nb0[t][:], xc[t][:, b:b + 1], r[:],
                        op0=mybir.AluOpType.add, op1=mybir.AluOpType.max)
                p0 = work.tile([128, HID], f32, tag="p0")
                nc.gpsimd.partition_all_reduce(p0[:], r[:], 128, ReduceOp.max)
                # all partitions of p0 hold the same reduced row; DMA one out
                # (engines can't start at arbitrary partitions, DMA can write them)
                nc.sync.dma_start(hrows[b:b + 1, :], p0[0:1, :])

              # ---- transpose -h rows into column tiles [128, BS] ----
              hcols = []
              for c in range(NT1):
                ps = psp.tile([128, BS], f32, tag="ps_t")
                nc.tensor.transpose(
                    ps[:], hrows[:, 128 * c:128 * (c + 1)], ident[:BS, :BS])
                hc = tabs.tile([128, BS], f32, tag=f"hc_{c}")
                nc.scalar.copy(hc[:], ps[:])
                hcols.append(hc)

              # ---- layer 1 ----
              for b in range(BS):
                r = work.tile([128, IN], f32, tag="r1")
                nc.vector.scalar_tensor_tensor(
                    r[:], b1[0][:], hcols[0][:, b:b + 1], init1[:],
                    op0=mybir.AluOpType.subtract, op1=mybir.AluOpType.max)
                for u in range(1, NT1):
                    nc.vector.scalar_tensor_tensor(
                        r[:], b1[u][:], hcols[u][:, b:b + 1], r[:],
                        op0=mybir.AluOpType.subtract, op1=mybir.AluOpType.max)
                p1 = work.tile([128, IN], f32, tag="p1")
                nc.gpsimd.partition_all_reduce(p1[:], r[:], 128, ReduceOp.max)
                nc.sync.dma_start(out[b:b + 1, :], p1[0:1, :])
    nc.compile()
    return nc


def _make_in_maps(x, sel0, sel1):
    x = np.ascontiguousarray(np.asarray(x, dtype=np.float32))
    nb0t = np.ascontiguousarray(
        np.where(np.asarray(sel0).T == 1, 0.0, -4.0).astype(np.float32))
    b1t = np.ascontiguousarray(
        np.where(np.asarray(sel1).T == 1, 0.0, -4.0).astype(np.float32))
    in_maps = []
    for c in range(NCORES):
        xs = x[c * BS:(c + 1) * BS, :]             # [BS, IN]
        xtn = np.ascontiguousarray((-xs).T)        # [IN, BS]
        in_maps.append({"xtn": xtn, "nb0t": nb0t, "b1t": b1t})
    return in_maps


def _fast_runner(nc):
    """Build a reusable jitted runner (same plumbing as
    bass2jax.run_bass_via_pjrt, but the jitted callable is cached so repeat
    kernel() calls don't re-trace/re-compile)."""
    import jax
    from jax.sharding import Mesh, PartitionSpec
    from jax.experimental.shard_map import shard_map
    import concourse.mybir as mb
    from concourse.bass2jax import (_bass_exec_p, install_neuronx_cc_hook,
                                    partition_id_tensor)

    install_neuronx_cc_hook()
    partition_name = (nc.partition_id_tensor.name
                      if nc.partition_id_tensor else None)
    in_names, out_names, out_avals = [], [], []
    for alloc in nc.m.functions[0].allocations:
        if not isinstance(alloc, mb.MemoryLocationSet):
            continue
        name = alloc.memorylocations[0].name
        if alloc.kind == "ExternalInput":
            if name != partition_name:
                in_names.append(name)
        elif alloc.kind == "ExternalOutput":
            out_names.append(name)
            out_avals.append(jax.core.ShapedArray(
                tuple(alloc.tensor_shape), mb.dt.np(alloc.dtype)))
    n_params = len(in_names)
    n_outs = len(out_avals)
    all_in_names = list(in_names) + list(out_names)
    if partition_name is not None:
        all_in_names.append(partition_name)
    donate = tuple(range(n_params, n_params + n_outs))

    def _body(*args):
        operands = list(args)
        if partition_name is not None:
            operands.append(partition_id_tensor())
        return tuple(_bass_exec_p.bind(
            *operands, out_avals=tuple(out_avals), in_names=tuple(all_in_names),
            out_names=tuple(out_names), lowering_input_output_aliases=(),
            sim_require_finite=True, sim_require_nnan=True, nc=nc))

    devices = jax.devices()[:NCORES]
    mesh = Mesh(np.asarray(devices), ("core",))
    # per-core inputs (xtn) are sharded along axis 0; the sel-derived bias
    # tables are identical on every core -> replicate them (one H2D transfer
    # instead of 8 concatenated copies through the relay)
    repl = {"nb0t", "b1t"}
    in_specs = tuple(
        (PartitionSpec() if nm in repl else PartitionSpec("core"))
        for nm in in_names) + (PartitionSpec("core"),) * n_outs
    sharded = jax.jit(
        shard_map(_body, mesh=mesh, in_specs=in_specs,
                  out_specs=(PartitionSpec("core"),) * n_outs,
                  check_rep=False),
        donate_argnums=donate, keep_unused=True)

    def run(in_maps):
        concat_in = [
            np.asarray(in_maps[0][nm]) if nm in repl else
            np.concatenate([np.asarray(in_maps[c][nm]) for c in range(NCORES)],
                           axis=0)
            for nm in in_names]
        concat_zeros = [
            np.zeros((NCORES * a.shape[0], *a.shape[1:]), a.dtype)
            for a in out_avals]
        out_arrs = sharded(*concat_in, *concat_zeros)
        return [
            {nm: np.asarray(out_arrs[i]).reshape(NCORES, *out_avals[i].shape)[c]
             for i, nm in enumerate(out_names)}
            for c in range(NCORES)]

    return run


def kernel(x, sel0, sel1):
    in_maps = _make_in_maps(x, sel0, sel1)
    if "nc" not in _cache:
        _cache["nc"] = _build()
        # first call goes through the stock runner (compiles the NEFF)
        res = bass_utils.run_bass_kernel_spmd(
            _cache["nc"], in_maps, core_ids=list(range(NCORES)))
        results = res.results
        _cache["run"] = _fast_runner(_cache["nc"])
    else:
        results = _cache["run"](in_maps)
    return np.concatenate([results[c]["out"] for c in range(NCORES)], axis=0)


# revision 13
# speedup vs baseline: 1246.2050x; 3.8389x over previous
"""DAAutoencoder (2-layer DAA net: masked min then masked max) on 8 TRN2 cores.

Math (reference):
  h[b,o]   = min over {i: sel0[o,i]==1} of x[b,i]   (2.0 if no edge)
  out[b,j] = max over {o: sel1[j,o]==1} of h[b,o]   (-1.0 if no edge)

Kernel formulation (exact, selection-only — bitwise identical to reference):
  Work in the negated domain for layer 0 so both layers reduce with max:
    -h[b,o]  = max( -2.0, max_i( (-x[b,i]) + nb0[i,o] ) ),  nb0 = 0 if edge else -4
    out[b,j] = max( -1.0, max_o( (-(-h[b,o])) + b1[o,j] ) ), b1 = 0 if edge else -4
  Masked-out lanes land below the init value and never win; winners are the
  original float values untouched (x + 0.0 == x exactly).

Data-parallel over batch: 256 rows -> 8 cores x 32 rows. sel-derived bias
tables replicated. Per core, per batch row b:
  layer 0: 8x scalar_tensor_tensor [128i,512o]  (op0=add per-partition -x col,
           op1=max into running acc) -> gpsimd partition_all_reduce(max)
           -> row copy -> (-h) rows [32,512]
  PE-transpose (-h) rows -> (-h) column tiles [128o,32b]
  layer 1: 4x scalar_tensor_tensor [128o,1024j] (op0=subtract (-h) col == add h,
           op1=max) -> partition_all_reduce(max) -> row copy -> out rows
Reduction dim always sits on partitions; the per-(b) vector enters as the
per-partition scalar operand of scalar_tensor_tensor, so nothing is ever
broadcast across partitions.
"""

import numpy as np

import concourse.bacc as bacc
import concourse.mybir as mybir
from concourse import tile, masks
from concourse.bass_isa import ReduceOp
from concourse import bass_utils

B, IN, HID = 256, 1024, 512
NCORES = 8
BS = B // NCORES  # 32 batch rows per core

_cache = {}


def _build(repeat=1):
    nc = bacc.Bacc("TRN2", target_bir_lowering=False, debug=False,
                   num_devices=NCORES)
    f32 = mybir.dt.float32
    xtn = nc.dram_tensor("xtn", [IN, BS], f32, kind="ExternalInput")
    nb0t = nc.dram_tensor("nb0t", [IN, HID], f32, kind="ExternalInput")
    b1t = nc.dram_tensor("b1t", [HID, IN], f32, kind="ExternalInput")
    out = nc.dram_tensor("out", [BS, IN], f32, kind="ExternalOutput")

    NT0 = IN // 128   # 8 i-tiles for layer 0
    NT1 = HID // 128  # 4 o-tiles for layer 1

    with tile.TileContext(nc) as tc:
        with tc.tile_pool(name="tables", bufs=1) as tabs, \
             tc.tile_pool(name="work", bufs=3) as work, \
             tc.tile_pool(name="res", bufs=1) as resp, \
             tc.tile_pool(name="psum", bufs=4, space="PSUM") as psp:

            nb0 = []
            for t in range(NT0):
                tt = tabs.tile([128, HID], f32, tag=f"nb0_{t}")
                nc.sync.dma_start(tt[:], nb0t[128 * t:128 * (t + 1), :])
                nb0.append(tt)
            b1 = []
            for u in range(NT1):
                tt = tabs.tile([128, IN], f32, tag=f"b1_{u}")
                nc.sync.dma_start(tt[:], b1t[128 * u:128 * (u + 1), :])
                b1.append(tt)
            xc = []
            for t in range(NT0):
                tt = tabs.tile([128, BS], f32, tag=f"x_{t}")
                nc.sync.dma_start(tt[:], xtn[128 * t:128 * (t + 1), :])
                xc.append(tt)

            init0 = tabs.tile([128, HID], f32, tag="init0")
            nc.vector.memset(init0[:], -2.0)
            init1 = tabs.tile([128, IN], f32, tag="init1")
            nc.vector.memset(init1[:], -1.0)
            ident = tabs.tile([128, 128], f32, tag="ident")
            masks.make_identity(nc, ident[:])

            hrows = resp.tile([BS, HID], f32, tag="hrows")     # rows of -h

            for _rep in range(repeat):
              # ---- layer 0 ----
              for b in range(BS):
                r = work.tile([128, HID], f32, tag="r0")
                nc.vector.scalar_tensor_tensor(
                    r[:], nb0[0][:], xc[0][:, b:b + 1], init0[:],
                    op0=mybir.AluOpType.add, op1=mybir.AluOpType.max)
                for t in range(1, NT0):
                    nc.vector.scalar_tensor_tensor(
                        r[:], nb0[t][:], xc[t][:, b:b + 1], r[:],
                        op0=mybir.AluOpType.add, op1=mybir.AluOpType.max)
                p0 = work.tile([128, HID], f32, tag="p0")
                nc.gpsimd.partition_all_reduce(p0[:], r[:], 128, ReduceOp.max)
                # all partitions of p0 hold the same reduced row; DMA one out
                # (engines can't start at arbitrary partitions, DMA can write them)
                nc.sync.dma_start(hrows[b:b + 1, :], p0[0:1, :])

              # ---- transpose -h rows into column tiles [128, BS] ----
              hcols = []
              for c in range(NT1):
                ps = psp.tile([128, BS], f32, tag="ps_t")
                nc.tensor.transpose(
                    ps[:], hrows[:, 128 * c:128 * (c + 1)], ident[:BS, :BS])
                hc = tabs.tile([128, BS], f32, tag=f"hc_{c}")
                nc.scalar.copy(hc[:], ps[:])
                hcols.append(hc)

              # ---- layer 1 ----
              for b in range(BS):
                r = work.tile([128, IN], f32, tag="r1")
                nc.vector.scalar_tensor_tensor(
                    r[:], b1[0][:], hcols[0][:, b:b + 1], init1[:],
                    op0=mybir.AluOpType.subtract, op1=mybir.AluOpType.max)
                for u in range(1, NT1):
                    nc.vector.scalar_tensor_tensor(
                        r[:], b1[u][:], hcols[u][:, b:b + 1], r[:],
                        op0=mybir.AluOpType.subtract, op1=mybir.AluOpType.max)
                p1 = work.tile([128, IN], f32, tag="p1")
                nc.gpsimd.partition_all_reduce(p1[:], r[:], 128, ReduceOp.max)
                nc.sync.dma_start(out[b:b + 1, :], p1[0:1, :])
    nc.compile()
    return nc


def _make_in_maps(x, sel0, sel1):
    x = np.ascontiguousarray(np.asarray(x, dtype=np.float32))
    nb0t = np.ascontiguousarray(
        np.where(np.asarray(sel0).T == 1, 0.0, -4.0).astype(np.float32))
    b1t = np.ascontiguousarray(
        np.where(np.asarray(sel1).T == 1, 0.0, -4.0).astype(np.float32))
    in_maps = []
    for c in range(NCORES):
        xs = x[c * BS:(c + 1) * BS, :]             # [BS, IN]
        xtn = np.ascontiguousarray((-xs).T)        # [IN, BS]
        in_maps.append({"xtn": xtn, "nb0t": nb0t, "b1t": b1t})
    return in_maps


def _fast_runner(nc, donate=True):
    """Build a reusable jitted runner (same plumbing as
    bass2jax.run_bass_via_pjrt, but the jitted callable is cached so repeat
    kernel() calls don't re-trace/re-compile).

    donate=False additionally keeps all inputs (including the zero output
    buffers) resident on device across calls — safe here because the kernel
    writes every output element — so repeat calls do no H2D at all."""
    import jax
    from jax.sharding import Mesh, PartitionSpec
    from jax.experimental.shard_map import shard_map
    import concourse.mybir as mb
    from concourse.bass2jax import (_bass_exec_p, install_neuronx_cc_hook,
                                    partition_id_tensor)

    install_neuronx_cc_hook()
    partition_name = (nc.partition_id_tensor.name
                      if nc.partition_id_tensor else None)
    in_names, out_names, out_avals = [], [], []
    for alloc in nc.m.functions[0].allocations:
        if not isinstance(alloc, mb.MemoryLocationSet):
            continue
        name = alloc.memorylocations[0].name
        if alloc.kind == "ExternalInput":
            if name != partition_name:
                in_names.append(name)
        elif alloc.kind == "ExternalOutput":
            out_names.append(name)
            out_avals.append(jax.core.ShapedArray(
                tuple(alloc.tensor_shape), mb.dt.np(alloc.dtype)))
    n_params = len(in_names)
    n_outs = len(out_avals)
    all_in_names = list(in_names) + list(out_names)
    if partition_name is not None:
        all_in_names.append(partition_name)
    donate_idx = tuple(range(n_params, n_params + n_outs))

    def _body(*args):
        operands = list(args)
        if partition_name is not None:
            operands.append(partition_id_tensor())
        return tuple(_bass_exec_p.bind(
            *operands, out_avals=tuple(out_avals), in_names=tuple(all_in_names),
            out_names=tuple(out_names), lowering_input_output_aliases=(),
            sim_require_finite=True, sim_require_nnan=True, nc=nc))

    devices = jax.devices()[:NCORES]
    mesh = Mesh(np.asarray(devices), ("core",))
    # per-core inputs (xtn) are sharded along axis 0; the sel-derived bias
    # tables are identical on every core -> replicate them (one H2D transfer
    # instead of 8 concatenated copies through the relay)
    repl = {"nb0t", "b1t"}
    in_specs = tuple(
        (PartitionSpec() if nm in repl else PartitionSpec("core"))
        for nm in in_names) + (PartitionSpec("core"),) * n_outs
    sharded = jax.jit(
        shard_map(_body, mesh=mesh, in_specs=in_specs,
                  out_specs=(PartitionSpec("core"),) * n_outs,
                  check_rep=False),
        donate_argnums=(donate_idx if donate else ()), keep_unused=True)

    dev_cache = {}

    def run(in_maps):
        concat_in = [
            np.asarray(in_maps[0][nm]) if nm in repl else
            np.concatenate([np.asarray(in_maps[c][nm]) for c in range(NCORES)],
                           axis=0)
            for nm in in_names]
        if donate:
            concat_zeros = [
                np.zeros((NCORES * a.shape[0], *a.shape[1:]), a.dtype)
                for a in out_avals]
        else:
            if "zeros" not in dev_cache:
                dev_cache["zeros"] = [
                    np.zeros((NCORES * a.shape[0], *a.shape[1:]), a.dtype)
                    for a in out_avals]
            concat_zeros = dev_cache["zeros"]
        out_arrs = sharded(*concat_in, *concat_zeros)
        return [
            {nm: np.asarray(out_arrs[i]).reshape(NCORES, *out_avals[i].shape)[c]
             for i, nm in enumerate(out_names)}
            for c in range(NCORES)]

    def run_dev(dev_args):
        return sharded(*dev_args)

    def prepare_dev(in_maps):
        from jax.sharding import NamedSharding
        host = ([np.asarray(in_maps[0][nm]) if nm in repl else
                 np.concatenate([np.asarray(in_maps[c][nm])
                                 for c in range(NCORES)], axis=0)
                 for nm in in_names]
                + [np.zeros((NCORES * a.shape[0], *a.shape[1:]), a.dtype)
                   for a in out_avals])
        return [jax.device_put(a, NamedSharding(mesh, s))
                for a, s in zip(host, in_specs)]

    run.prepare_dev = prepare_dev
    run.run_dev = run_dev
    return run


def kernel(x, sel0, sel1):
    in_maps = _make_in_maps(x, sel0, sel1)
    if "nc" not in _cache:
        _cache["nc"] = _build()
        # first call goes through the stock runner (compiles the NEFF)
        res = bass_utils.run_bass_kernel_spmd(
            _cache["nc"], in_maps, core_ids=list(range(NCORES)))
        results = res.results
        _cache["run"] = _fast_runner(_cache["nc"])
    else:
        results = _cache["run"](in_maps)
    return np.concatenate([results[c]["out"] for c in range(NCORES)], axis=0)


# revision 14
# speedup vs baseline: 1848.8786x; 1.4836x over previous
"""DAAutoencoder (2-layer DAA net: masked min then masked max) on 8 TRN2 cores.

Math (reference):
  h[b,o]   = min over {i: sel0[o,i]==1} of x[b,i]   (2.0 if no edge)
  out[b,j] = max over {o: sel1[j,o]==1} of h[b,o]   (-1.0 if no edge)

Kernel formulation (exact, selection-only — bitwise identical to reference):
  Work in the negated domain for layer 0 so both layers reduce with max:
    -h[b,o]  = max( -2.0, max_i( (-x[b,i]) + nb0[i,o] ) ),  nb0 = 0 if edge else -4
    out[b,j] = max( -1.0, max_o( (-(-h[b,o])) + b1[o,j] ) ), b1 = 0 if edge else -4
  Masked-out lanes land below the init value and never win; winners are the
  original float values untouched (x + 0.0 == x exactly).

Data-parallel over batch: 256 rows -> 8 cores x 32 rows. sel-derived bias
tables replicated. Per core, per batch row b:
  layer 0: 8x scalar_tensor_tensor [128i,512o]  (op0=add per-partition -x col,
           op1=max into running acc) -> gpsimd partition_all_reduce(max)
           -> row copy -> (-h) rows [32,512]
  PE-transpose (-h) rows -> (-h) column tiles [128o,32b]
  layer 1: 4x scalar_tensor_tensor [128o,1024j] (op0=subtract (-h) col == add h,
           op1=max) -> partition_all_reduce(max) -> row copy -> out rows
Reduction dim always sits on partitions; the per-(b) vector enters as the
per-partition scalar operand of scalar_tensor_tensor, so nothing is ever
broadcast across partitions.
"""

import numpy as np

import concourse.bacc as bacc
import concourse.mybir as mybir
from concourse import tile, masks
from concourse.bass_isa import ReduceOp
from concourse import bass_utils

B, IN, HID = 256, 1024, 512
NCORES = 8
BS = B // NCORES  # 32 batch rows per core

_cache = {}


def _build(repeat=1):
    nc = bacc.Bacc("TRN2", target_bir_lowering=False, debug=False,
                   num_devices=NCORES)
    f32 = mybir.dt.float32
    xtn = nc.dram_tensor("xtn", [IN, BS], f32, kind="ExternalInput")
    nb0t = nc.dram_tensor("nb0t", [IN, HID], f32, kind="ExternalInput")
    b1t = nc.dram_tensor("b1t", [HID, IN], f32, kind="ExternalInput")
    out = nc.dram_tensor("out", [BS, IN], f32, kind="ExternalOutput")

    NT0 = IN // 128   # 8 i-tiles for layer 0
    NT1 = HID // 128  # 4 o-tiles for layer 1

    with tile.TileContext(nc) as tc:
        with tc.tile_pool(name="tables", bufs=1) as tabs, \
             tc.tile_pool(name="work", bufs=6) as work, \
             tc.tile_pool(name="res", bufs=1) as resp, \
             tc.tile_pool(name="psum", bufs=4, space="PSUM") as psp:

            nb0 = []
            for t in range(NT0):
                tt = tabs.tile([128, HID], f32, tag=f"nb0_{t}")
                nc.sync.dma_start(tt[:], nb0t[128 * t:128 * (t + 1), :])
                nb0.append(tt)
            b1 = []
            for u in range(NT1):
                tt = tabs.tile([128, IN], f32, tag=f"b1_{u}")
                nc.sync.dma_start(tt[:], b1t[128 * u:128 * (u + 1), :])
                b1.append(tt)
            xc = []
            for t in range(NT0):
                tt = tabs.tile([128, BS], f32, tag=f"x_{t}")
                nc.sync.dma_start(tt[:], xtn[128 * t:128 * (t + 1), :])
                xc.append(tt)

            init0 = tabs.tile([128, HID], f32, tag="init0")
            nc.vector.memset(init0[:], -2.0)
            init1 = tabs.tile([128, IN], f32, tag="init1")
            nc.vector.memset(init1[:], -1.0)
            ident = tabs.tile([128, 128], f32, tag="ident")
            masks.make_identity(nc, ident[:])

            hrows = resp.tile([BS, HID], f32, tag="hrows")     # rows of -h

            for _rep in range(repeat):
              # ---- layer 0 ----
              for b in range(BS):
                r = work.tile([128, HID], f32, tag="r0")
                nc.vector.scalar_tensor_tensor(
                    r[:], nb0[0][:], xc[0][:, b:b + 1], init0[:],
                    op0=mybir.AluOpType.add, op1=mybir.AluOpType.max)
                for t in range(1, NT0):
                    nc.vector.scalar_tensor_tensor(
                        r[:], nb0[t][:], xc[t][:, b:b + 1], r[:],
                        op0=mybir.AluOpType.add, op1=mybir.AluOpType.max)
                p0 = work.tile([128, HID], f32, tag="p0")
                nc.gpsimd.partition_all_reduce(p0[:], r[:], 128, ReduceOp.max)
                # all partitions of p0 hold the same reduced row; DMA one out
                # (engines can't start at arbitrary partitions, DMA can write them)
                nc.sync.dma_start(hrows[b:b + 1, :], p0[0:1, :])

              # ---- transpose -h rows into column tiles [128, BS] ----
              hcols = []
              for c in range(NT1):
                ps = psp.tile([128, BS], f32, tag="ps_t")
                nc.tensor.transpose(
                    ps[:], hrows[:, 128 * c:128 * (c + 1)], ident[:BS, :BS])
                hc = tabs.tile([128, BS], f32, tag=f"hc_{c}")
                nc.scalar.copy(hc[:], ps[:])
                hcols.append(hc)

              # ---- layer 1 ----
              for b in range(BS):
                r = work.tile([128, IN], f32, tag="r1")
                nc.vector.scalar_tensor_tensor(
                    r[:], b1[0][:], hcols[0][:, b:b + 1], init1[:],
                    op0=mybir.AluOpType.subtract, op1=mybir.AluOpType.max)
                for u in range(1, NT1):
                    nc.vector.scalar_tensor_tensor(
                        r[:], b1[u][:], hcols[u][:, b:b + 1], r[:],
                        op0=mybir.AluOpType.subtract, op1=mybir.AluOpType.max)
                p1 = work.tile([128, IN], f32, tag="p1")
                nc.gpsimd.partition_all_reduce(p1[:], r[:], 128, ReduceOp.max)
                nc.sync.dma_start(out[b:b + 1, :], p1[0:1, :])
    nc.compile()
    return nc


def _make_in_maps(x, sel0, sel1):
    x = np.ascontiguousarray(np.asarray(x, dtype=np.float32))
    nb0t = np.ascontiguousarray(
        np.where(np.asarray(sel0).T == 1, 0.0, -4.0).astype(np.float32))
    b1t = np.ascontiguousarray(
        np.where(np.asarray(sel1).T == 1, 0.0, -4.0).astype(np.float32))
    in_maps = []
    for c in range(NCORES):
        xs = x[c * BS:(c + 1) * BS, :]             # [BS, IN]
        xtn = np.ascontiguousarray((-xs).T)        # [IN, BS]
        in_maps.append({"xtn": xtn, "nb0t": nb0t, "b1t": b1t})
    return in_maps


def _fast_runner(nc, donate=True):
    """Build a reusable jitted runner (same plumbing as
    bass2jax.run_bass_via_pjrt, but the jitted callable is cached so repeat
    kernel() calls don't re-trace/re-compile).

    donate=False additionally keeps all inputs (including the zero output
    buffers) resident on device across calls — safe here because the kernel
    writes every output element — so repeat calls do no H2D at all."""
    import jax
    from jax.sharding import Mesh, PartitionSpec
    from jax.experimental.shard_map import shard_map
    import concourse.mybir as mb
    from concourse.bass2jax import (_bass_exec_p, install_neuronx_cc_hook,
                                    partition_id_tensor)

    install_neuronx_cc_hook()
    partition_name = (nc.partition_id_tensor.name
                      if nc.partition_id_tensor else None)
    in_names, out_names, out_avals = [], [], []
    for alloc in nc.m.functions[0].allocations:
        if not isinstance(alloc, mb.MemoryLocationSet):
            continue
        name = alloc.memorylocations[0].name
        if alloc.kind == "ExternalInput":
            if name != partition_name:
                in_names.append(name)
        elif alloc.kind == "ExternalOutput":
            out_names.append(name)
            out_avals.append(jax.core.ShapedArray(
                tuple(alloc.tensor_shape), mb.dt.np(alloc.dtype)))
    n_params = len(in_names)
    n_outs = len(out_avals)
    all_in_names = list(in_names) + list(out_names)
    if partition_name is not None:
        all_in_names.append(partition_name)
    donate_idx = tuple(range(n_params, n_params + n_outs))

    def _body(*args):
        operands = list(args)
        if partition_name is not None:
            operands.append(partition_id_tensor())
        return tuple(_bass_exec_p.bind(
            *operands, out_avals=tuple(out_avals), in_names=tuple(all_in_names),
            out_names=tuple(out_names), lowering_input_output_aliases=(),
            sim_require_finite=True, sim_require_nnan=True, nc=nc))

    devices = jax.devices()[:NCORES]
    mesh = Mesh(np.asarray(devices), ("core",))
    # per-core inputs (xtn) are sharded along axis 0; the sel-derived bias
    # tables are identical on every core -> replicate them (one H2D transfer
    # instead of 8 concatenated copies through the relay)
    repl = {"nb0t", "b1t"}
    in_specs = tuple(
        (PartitionSpec() if nm in repl else PartitionSpec("core"))
        for nm in in_names) + (PartitionSpec("core"),) * n_outs
    sharded = jax.jit(
        shard_map(_body, mesh=mesh, in_specs=in_specs,
                  out_specs=(PartitionSpec("core"),) * n_outs,
                  check_rep=False),
        donate_argnums=(donate_idx if donate else ()), keep_unused=True)

    dev_cache = {}

    def run(in_maps):
        concat_in = [
            np.asarray(in_maps[0][nm]) if nm in repl else
            np.concatenate([np.asarray(in_maps[c][nm]) for c in range(NCORES)],
                           axis=0)
            for nm in in_names]
        if donate:
            concat_zeros = [
                np.zeros((NCORES * a.shape[0], *a.shape[1:]), a.dtype)
                for a in out_avals]
        else:
            if "zeros" not in dev_cache:
                dev_cache["zeros"] = [
                    np.zeros((NCORES * a.shape[0], *a.shape[1:]), a.dtype)
                    for a in out_avals]
            concat_zeros = dev_cache["zeros"]
        out_arrs = sharded(*concat_in, *concat_zeros)
        return [
            {nm: np.asarray(out_arrs[i]).reshape(NCORES, *out_avals[i].shape)[c]
             for i, nm in enumerate(out_names)}
            for c in range(NCORES)]

    def run_dev(dev_args):
        return sharded(*dev_args)

    def prepare_dev(in_maps):
        from jax.sharding import NamedSharding
        host = ([np.asarray(in_maps[0][nm]) if nm in repl else
                 np.concatenate([np.asarray(in_maps[c][nm])
                                 for c in range(NCORES)], axis=0)
                 for nm in in_names]
                + [np.zeros((NCORES * a.shape[0], *a.shape[1:]), a.dtype)
                   for a in out_avals])
        return [jax.device_put(a, NamedSharding(mesh, s))
                for a, s in zip(host, in_specs)]

    run.prepare_dev = prepare_dev
    run.run_dev = run_dev
    return run


def kernel(x, sel0, sel1):
    in_maps = _make_in_maps(x, sel0, sel1)
    if "nc" not in _cache:
        _cache["nc"] = _build()
        # first call goes through the stock runner (compiles the NEFF)
        res = bass_utils.run_bass_kernel_spmd(
            _cache["nc"], in_maps, core_ids=list(range(NCORES)))
        results = res.results
        _cache["run"] = _fast_runner(_cache["nc"])
    else:
        results = _cache["run"](in_maps)
    return np.concatenate([results[c]["out"] for c in range(NCORES)], axis=0)
